# revision 47
# baseline (speedup 1.0000x reference)
# BertSelfAttention TRN2 Bass kernel (v2).
#
# Full-input contract: kernel(**inputs) takes the unsharded tensors and
# returns the full [2, 2048, 1024] output. Internally shards across 8
# NeuronCores: core c handles batch c//4 and heads 4*(c%4) .. 4*(c%4)+3
# (data parallel over batch x tensor parallel over heads; no cross-core
# communication, host gathers).
#
# v2 changes vs v1 (671us stated baseline / ~256us remeasured; v2
# measures ~185-240us depending on session noise):
#  - exp split across two engines: per kc one [128,512] S half gets exact
#    exp on ScalarE, the other a Schraudolph fast-exp on DVE (int16
#    bitcast, immediate scalars; rel err ~1e-2 < 2e-2 gate). Roles
#    alternate by kc so every head sees half exact exp.
#  - S psum in per-head [128,512] halves; kc loop software-pipelined by
#    one step (S(kc+1) before PV(kc)) to hide producer latency.
#  - Q/K bias folded into the Act-engine psum->SBUF proj copies
#    (Identity activation with per-partition bias AP).
#  - loads via gpsimd cast-DMA (fp32->fp16) positioned on the Pool DMA
#    queue in dependency order (completions are counting semaphores).
#  - X.T/W.T stay on the PE (XBAR dma_start_transpose measured slower:
#    SP/Act hwdge queues are depth-0); copies alternate DVE/Act.
#  - V projection interleaved into the first attention block's kc loop;
#    V-proj psum shares one PSUM bank with the drain transposes.
#
# HW lessons (vs the TimelineSim cost model):
#  - GPSIMD/Pool cannot access PSUM at all (sim does not model this).
#  - DVE tensor_scalar with per-partition scalar AP (TensorScalarPtr) is
#    several times slower than with immediate scalars.
#  - fp16 DoublePixel matmul perf mode: bit-identical results but slower.
#  - fp8 DoubleRow would double matmul throughput but e4m3 quantization
#    of P/V/QK measures 2.5-3e-2 end-to-end: over the 2e-2 gate.
#
# Per-core dataflow (fp16 matmul operands, fp32 PSUM):
#   xn [tok,4,1024] fp16  (gpsimd cast-DMA from fp32 DRAM)
#   XT [i_w, tb, kk, tok_w] via PE transposes; WT [i_w, jj, kk, d_w]
#   QT/KT = WT.T @ XT   [d, 2048] per head pair, bias on Act copy
#   V natural [tok, d] + ones column -> Vt [128,16,4,65] interleaved
#   per (q-block 512, head-pair, key-chunk 128):
#     S.T = K @ Q.T   2 row-packed matmuls -> 2x psum [128 keys, 512]
#     P.T = exp(0.125*S.T + mask[key])  ScalarE exact / DVE fast-exp
#     C.T += V_aug.T @ P.T  -> psum [65, 512]; row 64 = denominator
#   drain: PE transpose C.T chunks -> [128 q, 65]; DVE reciprocal of
#   col 64, per-partition scale of cols 0..63 -> OUT -> DMA.

import numpy as np

from concourse import bacc
import concourse.mybir as mybir
import concourse.tile as tile
from concourse.bass import ds, ts
from concourse.bass_utils import run_bass_kernel_spmd
from concourse.masks import make_identity

P = 128
L = 2048  # tokens per batch element
HF = 1024  # model width
DC = 256  # head dims per core (4 heads x 64)
F32 = mybir.dt.float32
DT = mybir.dt.float16  # matmul operand dtype (PSUM accumulation stays fp32)
I16 = mybir.dt.int16
EXP = mybir.ActivationFunctionType.Exp
IDN = mybir.ActivationFunctionType.Identity

# Schraudolph fast-exp on Pool: exp(0.125*s + m) ~= bitcast_fp16(int16(
#   s*SCH_A + m*SCH_M + SCH_B)); rel err ~3% per element, ~1.3e-2 end to
# end (harness gate is 2e-2). Offloads half the exp tiles from ScalarE.
SCH_A = 0.125 * 1024.0 / float(np.log(2.0))  # 184.664
SCH_M = 1024.0 / float(np.log(2.0))
SCH_B = 15.0 * 1024.0 - 45.0


def _emit(tc, x, wq, wk, wv, bq, bk, bv, mask, out, phases="all", mask_zero=True):
    nc = tc.nc
    from contextlib import ExitStack

    with ExitStack() as es:
        consts = es.enter_context(tc.tile_pool(name="consts", bufs=1))
        wtp = es.enter_context(tc.tile_pool(name="wt", bufs=1))
        qkvp = es.enter_context(tc.tile_pool(name="qkv", bufs=1))
        ldp = es.enter_context(tc.tile_pool(name="ld", bufs=1))

        # ---- consts ----
        ident = consts.tile([P, P], F32)
        make_identity(nc, ident)
        ident16 = consts.tile([P, P], DT)
        nc.vector.tensor_copy(ident16, ident)
        ones_f32 = consts.tile([1, P], F32)
        nc.gpsimd.memset(ones_f32, 1.0)
        ones_row = consts.tile([1, P], DT)
        nc.vector.tensor_copy(ones_row, ones_f32)
        # consts go over the gpsimd software-DGE path: the two hwdge queues
        # (SP, Act) are depth-0 (~4-6us per DMA round trip) and are needed
        # for the X.T / W.T XBAR transposes. The DMAs are emitted inside the
        # load sequence below (Pool-queue completions are a counting
        # semaphore, so queue position = readiness time).
        # q/k biases as per-partition columns [128, 2] (jj = partition group)
        bcol = {}
        for name in ("q", "k"):
            bcol[name] = consts.tile([P, 2], F32, tag=f"bc{name}", name=f"bc{name}")
        mask_sb = consts.tile([P, 16], F32)
        bv_f32 = consts.tile([1, DC], F32)
        bv_sb = consts.tile([1, DC], DT)

        # persistent per-core tensors
        QT = [qkvp.tile([P, L], DT, tag=f"qt{j}", name=f"qt{j}") for j in range(2)]
        KT = [qkvp.tile([P, L], DT, tag=f"kt{j}", name=f"kt{j}") for j in range(2)]
        # XT layout [i_within, token_block, i_block, token_within] so that one
        # XBAR transpose per 256-token chunk writes a contiguous 3D view
        XT = qkvp.tile([P, 16, 8, P], DT, tag="xt")
        # V stored interleaved per head: 65 slots (64 dims + ones column)
        Vt = qkvp.tile([P, 16, 260], DT, tag="v")
        Vt4 = Vt.rearrange("p t (h c) -> p t h c", c=65)
        ones64 = consts.tile([P, 64], F32)
        nc.gpsimd.memset(ones64, 1.0)
        nc.vector.tensor_copy(
            Vt4[:, :, :, 64], ones64.rearrange("p (t h) -> p t h", h=4)
        )

        # ---- input loads: gpsimd cast-DMA fp32->fp16 in small chunks ----
        # (128 descriptors each so the SWDGE ring never throttles; DMA time
        # is charged on fp16 output bytes, half of an fp32 hwdge load)
        wn = {}
        xn = []
        wmap = {"q": wq, "k": wk, "v": wv}

        def load_w(name):
            t = ldp.tile([P, 2, HF], DT, tag=f"wn{name}", name=f"wn{name}")
            nc.gpsimd.dma_start(t, wmap[name].rearrange("(j p) i -> p j i", p=P))
            wn[name] = t

        def load_x(xi):  # 512-token chunk
            t = ldp.tile([P, 4, HF], DT, tag=f"xn{xi}", name=f"xn{xi}")
            nc.gpsimd.dma_start(
                t, x[ds(512 * xi, 512), :].rearrange("(t p) i -> p t i", p=P)
            )
            xn.append(t)

        # ---- loads (gpsimd software DGE; queue position = readiness) ----
        load_x(0)
        load_w("k")
        nc.gpsimd.dma_start(bcol["q"], bq.rearrange("(j p) -> p j", p=P))
        nc.gpsimd.dma_start(bcol["k"], bk.rearrange("(j p) -> p j", p=P))
        load_x(1)
        load_w("q")
        load_x(2)
        nc.gpsimd.dma_start(mask_sb, mask.rearrange("(t p) -> p t", p=P))
        nc.gpsimd.dma_start(bv_f32, bv[None, :])
        load_x(3)
        load_w("v")
        nc.vector.tensor_copy(bv_sb, bv_f32)
        # per-key bias column for the Pool fast-exp path
        mask2_sb = consts.tile([P, 16], F32)
        nc.vector.tensor_scalar(
            mask2_sb, mask_sb, SCH_M, SCH_B, mybir.AluOpType.mult, mybir.AluOpType.add
        )

        # ---- W.T / X.T on the PE (hwdge queues are depth-0, ~10us/DMA
        # round trip in practice — unusable for the 20+ transposes needed).
        # psum->SBUF copies alternate DVE/Pool to share the drain load.
        WT = {}
        for name in ("k", "q", "v"):
            WT[name] = wtp.tile([P, 2, 8, P], DT, tag=f"wt{name}", name=f"wt{name}")

        with (
            tc.tile_pool(name="tpsB", bufs=4, space="PSUM") as tpsB,
            tc.tile_pool(name="pps", bufs=2, space="PSUM") as pps,
        ):

            def tp_w(name):
                for jj in range(2):
                    for kk in range(8):
                        pt = tpsB.tile([P, P], DT, tag="tpB", name="tpB")
                        nc.tensor.transpose(
                            pt, wn[name][:, jj, ts(kk, P)], ident16
                        )
                        if kk % 4:
                            nc.vector.tensor_copy(WT[name][:, jj, kk, :], pt)
                        else:
                            nc.scalar.copy(WT[name][:, jj, kk, :], pt)

            def tp_x(xi):
                for tt in range(4):
                    for kk in range(8):
                        pt = tpsB.tile([P, P], DT, tag="tpB", name="tpB")
                        nc.tensor.transpose(
                            pt, xn[xi][:, tt, ts(kk, P)], ident16
                        )
                        if kk % 4:
                            nc.vector.tensor_copy(XT[:, 4 * xi + tt, kk, :], pt)
                        else:
                            nc.scalar.copy(XT[:, 4 * xi + tt, kk, :], pt)

            def proj(name, Tarr, qc):
                for jj in range(2):
                    ps = pps.tile([P, 512], F32, tag="pp")
                    for it in range(8):
                        nc.tensor.matmul(
                            ps,
                            WT[name][:, jj, it, :],
                            XT[:, ds(4 * qc, 4), it, :],
                            start=(it == 0),
                            stop=(it == 7),
                        )
                    nc.scalar.activation(
                        Tarr[jj][:, ts(qc, 512)],
                        ps,
                        IDN,
                        bias=bcol[name][:, jj : jj + 1],
                        scale=1.0,
                    )

            tp_x(0)
            tp_w("k")
            proj("k", KT, 0)
            tp_w("q")
            proj("q", QT, 0)
            tp_x(1)
            proj("k", KT, 1)
            proj("q", QT, 1)
            tp_w("v")
            tp_x(2)
            proj("k", KT, 2)
            proj("q", QT, 2)
            tp_x(3)
            proj("k", KT, 3)
            proj("q", QT, 3)

        if phases == "front":
            dummy = consts.tile([P, DC], F32, tag="dummy", name="dummy")
            nc.vector.tensor_copy(dummy, QT[0][:, 0:DC].bitcast(F32))
            nc.sync.dma_start(out[0:P, :], dummy)
            return

        # ---- attention, V projection interleaved into (qb=0, j=0) ----
        with (
            tc.tile_pool(name="ptp", bufs=4) as ptp,
            tc.tile_pool(name="cts", bufs=2) as ctsp,
            tc.tile_pool(name="rcpp", bufs=2) as rcpp,
            tc.tile_pool(name="outp", bufs=2) as outp,
            tc.tile_pool(name="stps", bufs=4, space="PSUM") as stps,
            tc.tile_pool(name="ctps", bufs=3, space="PSUM") as ctps,
            tc.tile_pool(name="misc", bufs=1, space="PSUM") as misc,
        ):
            # one shared PSUM bank rotates between V-proj psums (qb0/j0 only)
            # and drain-transpose psums (strictly after the last V-proj copy)

            def vproj(vc):
                ps = misc.tile([P, 512], F32, tag="mp", name="mp")[:, 0:DC]
                for it in range(8):
                    nc.tensor.matmul(
                        ps,
                        XT[:, vc, it, :],
                        WT["v"][:, :, it, :],
                        start=(it == 0),
                        stop=False,
                    )
                nc.tensor.matmul(
                    ps, ones_row[0:1, :], bv_sb[0:1, :], start=False, stop=True
                )
                nc.scalar.copy(
                    Vt4[:, vc, :, 0:64], ps.rearrange("p (h c) -> p h c", c=64)
                )

            for qb in range(4):
                OUT = outp.tile([P, 4, DC], F32, tag="out")
                for j in range(2):  # head pair (heads 2j, 2j+1)
                    CT = [
                        ctps.tile([65, 512], F32, tag="ct", name=f"ct{qb}_{j}_{hl}")
                        for hl in range(2)
                    ]

                    def s_and_p(kc):
                        # S in per-head [128,512] psum halves so the two exp
                        # producers (Act for h0; DVE/Pool alternating for h1)
                        # run concurrently each kc step
                        if qb == 0 and j == 0:
                            vproj(kc)
                        pts = [None, None]
                        for hl in range(2):
                            stt = stps.tile([P, 512], F32, tag="st", name="st")
                            nc.tensor.matmul(
                                stt,
                                KT[j][ts(hl, 64), ts(kc, P)],
                                QT[j][ts(hl, 64), ts(qb, 512)],
                                start=True,
                                stop=True,
                                tile_position=(64 * hl, 0),
                            )
                            pt = ptp.tile([P, 512], DT, tag=f"pt{hl}", name="pt")
                            if hl != kc % 2:  # fast-exp on DVE (GPSIMD
                                # cannot read PSUM on HW); role alternates by
                                # kc so every head sees half exact-exp,
                                # halving the max error. Immediate scalars
                                # (TensorScalar, not the slower Ptr variant)
                                # when the mask is all-zeros; per-partition
                                # scalar AP otherwise.
                                sc2 = (
                                    SCH_B
                                    if mask_zero
                                    else mask2_sb[:, kc : kc + 1]
                                )
                                nc.vector.tensor_scalar(
                                    pt.bitcast(I16),
                                    stt,
                                    SCH_A,
                                    sc2,
                                    mybir.AluOpType.mult,
                                    mybir.AluOpType.add,
                                )
                            else:
                                nc.scalar.activation(
                                    pt,
                                    stt,
                                    EXP,
                                    bias=mask_sb[:, kc : kc + 1],
                                    scale=0.125,
                                )
                            pts[hl] = pt
                        return pts

                    def pv(kc, pts):
                        for hl in range(2):
                            h = 2 * j + hl
                            nc.tensor.matmul(
                                CT[hl],
                                Vt4[:, kc, h, :],
                                pts[hl],
                                start=(kc == 0),
                                stop=(kc == 15),
                            )

                    # software-pipelined by one kc step: S(kc+1) is emitted
                    # before PV(kc) so the exp latency is hidden from the PE
                    prev = s_and_p(0)
                    for kc in range(1, 16):
                        cur = s_and_p(kc)
                        pv(kc - 1, prev)
                        prev = cur
                    pv(15, prev)
                    for hl in range(2):
                        h = 2 * j + hl
                        cs = ctsp.tile([65, 512], F32, tag="cts")
                        nc.vector.tensor_copy(cs, CT[hl])
                        for cc in range(4):
                            # the final drain can pipeline through the freed
                            # S-psum buffers instead of the single misc bank
                            pool = stps if (qb == 3 and j == 1) else misc
                            tag = "st" if (qb == 3 and j == 1) else "mp"
                            tp = pool.tile([P, 512], F32, tag=tag, name="tp")[:, 0:65]
                            nc.tensor.transpose(tp, cs[:, ts(cc, P)], ident[0:65, 0:65])
                            rcp = rcpp.tile([P, 1], F32, tag="rcp")
                            nc.vector.reciprocal(rcp, tp[:, 64:65])
                            nc.vector.tensor_scalar_mul(
                                OUT[:, cc, ts(h, 64)], tp[:, 0:64], rcp
                            )
                nc.sync.dma_start(
                    out[ds(512 * qb, 512), :].rearrange("(c p) d -> p c d", p=P), OUT
                )


def build_program(repeat=1, phases="all", loop=False, mask_zero=True):
    nc = bacc.Bacc("TRN2")
    x = nc.dram_tensor("x", [L, HF], F32, kind="ExternalInput").ap()
    wq = nc.dram_tensor("wq", [DC, HF], F32, kind="ExternalInput").ap()
    wk = nc.dram_tensor("wk", [DC, HF], F32, kind="ExternalInput").ap()
    wv = nc.dram_tensor("wv", [DC, HF], F32, kind="ExternalInput").ap()
    bq = nc.dram_tensor("bq", [DC], F32, kind="ExternalInput").ap()
    bk = nc.dram_tensor("bk", [DC], F32, kind="ExternalInput").ap()
    bv = nc.dram_tensor("bv", [DC], F32, kind="ExternalInput").ap()
    mask = nc.dram_tensor("mask", [L], F32, kind="ExternalInput").ap()
    out = nc.dram_tensor("out", [L, DC], F32, kind="ExternalOutput").ap()
    with tile.TileContext(nc) as tc:
        if loop and repeat > 1:
            with tc.For_i(0, repeat, 1):
                _emit(tc, x, wq, wk, wv, bq, bk, bv, mask, out, phases=phases,
                      mask_zero=mask_zero)
        else:
            for _rep in range(repeat):
                _emit(tc, x, wq, wk, wv, bq, bk, bv, mask, out, phases=phases,
                      mask_zero=mask_zero)
    nc.compile()
    return nc


_PROGS = {}


def _get_prog(repeat=1, phases="all", loop=False, mask_zero=True):
    key = (repeat, phases, loop, mask_zero)
    if key not in _PROGS:
        _PROGS[key] = build_program(repeat, phases, loop, mask_zero)
    return _PROGS[key]


def make_in_maps(hidden_states, attention_mask, Wq, bq, Wk, bk, Wv, bv):
    hs = np.ascontiguousarray(np.asarray(hidden_states, dtype=np.float32))
    am = np.asarray(attention_mask, dtype=np.float32)
    Wq, Wk, Wv = (np.asarray(w, dtype=np.float32) for w in (Wq, Wk, Wv))
    bq, bk, bv = (np.asarray(b, dtype=np.float32) for b in (bq, bk, bv))
    in_maps = []
    for c in range(8):
        b, g = divmod(c, 4)
        sl = slice(DC * g, DC * (g + 1))
        in_maps.append(
            {
                "x": hs[b],
                "wq": np.ascontiguousarray(Wq[sl]),
                "wk": np.ascontiguousarray(Wk[sl]),
                "wv": np.ascontiguousarray(Wv[sl]),
                "bq": np.ascontiguousarray(bq[sl]),
                "bk": np.ascontiguousarray(bk[sl]),
                "bv": np.ascontiguousarray(bv[sl]),
                "mask": np.ascontiguousarray(am[b, 0, 0, :]),
            }
        )
    return in_maps


def run_cores(in_maps, trace=False, **kw):
    mz = all(not m["mask"].any() for m in in_maps)
    nc = _get_prog(mask_zero=mz)
    return run_bass_kernel_spmd(nc, in_maps, list(range(8)), trace=trace, **kw)


def assemble(results):
    out = np.empty((2, L, HF), dtype=np.float32)
    for c in range(8):
        b, g = divmod(c, 4)
        out[b, :, DC * g : DC * (g + 1)] = results[c]["out"]
    return out


def kernel(hidden_states, attention_mask, Wq, bq, Wk, bk, Wv, bv):
    in_maps = make_in_maps(hidden_states, attention_mask, Wq, bq, Wk, bk, Wv, bv)
    res = run_cores(in_maps)
    return assemble(res.results)
